# revision 7
# baseline (speedup 1.0000x reference)
"""GumbelVQ tokenizer forward on 8 Trainium2 NeuronCores.

Math (per token row t, d=128, C=512 codes):
  x1 = x + (1-mask)*EPS
  xhat = x1 / max(||x1||, EPS)
  ab = xhat @ cb.T                      # (C,)
  indices = argmax(ab)                  # == argmin(2 - 2*ab)
  logits = 2*ab + noise                 # constant -2 dropped (softmax invariant)
  enc = softmax(logits)
  q = enc @ cb

Sharding: data-parallel over the token axis, 16384 tokens per core, no
communication.  The codebook (512x128) is replicated.

Per-core schedule: 16 "supers" of 8 tiles x 128 tokens (tokens on SBUF
partitions).  Engine assignment per tile:
  PE    : x1 transpose, mm1 (x1T.T @ cbT -> ab_raw), 4x enc transpose,
          mm2 (encT.T @ cb chunks -> q)
  DVE   : ssq reduce, fused PSUM-escape+scale+rowmax (tensor_scalar accum),
          max_index argmax, x1T escape copy
  ACT   : Square, Ln/Exp (rsqrt via exp(-0.5 ln x)), Exp+accum (softmax sum),
          second Exp with bias=-log(s) (normalized encodings), encT escape
          -- all funcs live in the single 'natural_log_exp_and_others' table
  GPSIMD: nudge add (broadcast), logits add, small copies
"""

import sys

sys.path.insert(0, "/opt/trn_rl_repo")

import numpy as np

N_CORES = 8
N_TOTAL = 131072
D = 128
C = 512
EPS = 1e-6
P = 128          # SBUF partitions = tokens per tile
G = 8            # tiles per super
LN2 = 0.6931471805599453

_CACHE = {}


def build_nc(R):
    """Build the per-core Bass module for R tokens (R % (P*G) == 0)."""
    import concourse.bacc as bacc
    import concourse.tile as tile
    import concourse.mybir as mybir

    dt = mybir.dt
    f32 = dt.float32
    Fn = mybir.ActivationFunctionType
    Op = mybir.AluOpType
    Ax = mybir.AxisListType

    assert R % (P * G) == 0
    S = R // (P * G)  # number of supers

    nc = bacc.Bacc(None)

    x_d = nc.dram_tensor("x", [R, D], f32, kind="ExternalInput")
    mask_d = nc.dram_tensor("mask", [R], f32, kind="ExternalInput")
    noise_d = nc.dram_tensor("noise", [R, C], f32, kind="ExternalInput")
    cbT_d = nc.dram_tensor("cbT", [D, C], f32, kind="ExternalInput")
    cb_d = nc.dram_tensor("cb", [C, D], f32, kind="ExternalInput")
    id_d = nc.dram_tensor("ident", [P, P], f32, kind="ExternalInput")
    q_d = nc.dram_tensor("quantized", [R, D], f32, kind="ExternalOutput")
    enc_d = nc.dram_tensor("encodings", [R, C], f32, kind="ExternalOutput")
    idx_d = nc.dram_tensor("indices", [R], dt.uint32, kind="ExternalOutput")

    with tile.TileContext(nc) as tc:
        with (
            tc.tile_pool(name="const", bufs=1) as const,
            tc.tile_pool(name="sup", bufs=2) as sup,
            tc.tile_pool(name="work", bufs=3) as work,
            tc.tile_pool(name="small", bufs=4) as small,
            tc.tile_pool(name="psum", bufs=2, space="PSUM") as psum,
        ):
            cbT_s = const.tile([D, C], f32)
            nc.sync.dma_start(cbT_s[:], cbT_d[:])
            cb_s = const.tile([P, 4, D], f32)
            nc.sync.dma_start(cb_s[:], cb_d[:].rearrange("(k c) d -> c k d", c=P))
            id_s = const.tile([P, P], f32)
            nc.sync.dma_start(id_s[:], id_d[:])
            ln2_s = const.tile([P, 1], f32)
            nc.gpsimd.memset(ln2_s[:], LN2)

            for s in range(S):
                t0 = s * G * P
                tok = slice(t0, t0 + G * P)

                x_sb = sup.tile([P, G, D], f32, tag="x_sb")
                nc.sync.dma_start(
                    x_sb[:], x_d[tok, :].rearrange("(g p) d -> p g d", p=P)
                )
                noise_sb = sup.tile([P, G, C], f32, tag="noise_sb")
                nc.sync.dma_start(
                    noise_sb[:], noise_d[tok, :].rearrange("(g p) c -> p g c", p=P)
                )
                mask_sb = small.tile([P, G], f32, tag="mask_sb")
                nc.sync.dma_start(
                    mask_sb[:], mask_d[tok].rearrange("(g p) -> p g", p=P)
                )

                enc_sb = sup.tile([P, G, C], f32, tag="enc_sb")
                q_sb = sup.tile([P, G, D], f32, tag="q_sb")
                idx_sb = small.tile([P, G], dt.uint32, tag="idx_sb")

                # nudge = (1-mask)*EPS ; x1 = x + nudge
                nudgev = small.tile([P, G], f32, tag="nudgev")
                nc.scalar.activation(
                    nudgev[:], mask_sb[:], Fn.Copy, bias=EPS, scale=-EPS
                )
                x1 = sup.tile([P, G, D], f32, tag="x1")
                nc.gpsimd.tensor_tensor(
                    x1[:],
                    x_sb[:],
                    nudgev[:].broadcast_to((P, G, D)),
                    op=Op.add,
                )

                # inv2 = 2 / max(sqrt(ssq), EPS) == exp(-0.5*ln(max(ssq,EPS^2)) + ln2)
                xsq = sup.tile([P, G, D], f32, tag="xsq")
                nc.scalar.activation(xsq[:], x1[:], Fn.Square)
                ssq = small.tile([P, G], f32, tag="ssq")
                nc.vector.reduce_sum(ssq[:], xsq[:], axis=Ax.X)
                ssqc = small.tile([P, G], f32, tag="ssqc")
                nc.vector.tensor_scalar_max(ssqc[:], ssq[:], EPS * EPS)
                lssq = small.tile([P, G], f32, tag="lssq")
                nc.scalar.activation(lssq[:], ssqc[:], Fn.Ln)
                inv2 = small.tile([P, G], f32, tag="inv2")
                nc.scalar.activation(inv2[:], lssq[:], Fn.Exp, bias=ln2_s[:], scale=-0.5)

                for g in range(G):
                    # x1T = x1[:,g,:]^T  (PE transpose, then escape to SBUF)
                    x1T_ps = psum.tile([P, D], f32, tag="x1T_ps")
                    nc.tensor.matmul(
                        x1T_ps[:], x1[:, g, :], id_s[:],
                        is_transpose=True, start=True, stop=True,
                    )
                    x1T_sb = work.tile([P, D], f32, tag="x1T_sb")
                    nc.vector.tensor_copy(x1T_sb[:], x1T_ps[:])

                    # ab_raw = x1 @ cbT   (PSUM [t, C])
                    ab_ps = psum.tile([P, C], f32, tag="ab_ps")
                    nc.tensor.matmul(
                        ab_ps[:], x1T_sb[:], cbT_s[:], start=True, stop=True
                    )

                    # ab2 = ab_raw * inv2[t]  (escape + scale + rowmax in one op)
                    ab2 = work.tile([P, C], f32, tag="ab2")
                    m1 = small.tile([P, 1], f32, tag="m1")
                    nc.vector.tensor_scalar(
                        ab2[:], ab_ps[:], inv2[:, g : g + 1], None,
                        op0=Op.mult, op1=Op.max, accum_out=m1[:],
                    )

                    # argmax: replicate rowmax into 8 slots, find its index
                    m8 = work.tile([P, 8], f32, tag="m8")
                    nc.vector.tensor_copy(m8[:], m1[:].broadcast_to((P, 8)))
                    idx8 = work.tile([P, 8], dt.uint32, tag="idx8")
                    nc.vector.max_index(idx8[:], m8[:], ab2[:])
                    nc.gpsimd.tensor_copy(idx_sb[:, g : g + 1], idx8[:, 0:1])

                    # logits = ab2 + noise  (no max-subtraction: bounded above)
                    logits = work.tile([P, C], f32, tag="logits")
                    nc.gpsimd.tensor_tensor(
                        logits[:], ab2[:], noise_sb[:, g, :], op=Op.add
                    )

                    # e = exp(logits), s = sum(e); enc = exp(logits - log s)
                    e_t = work.tile([P, C], f32, tag="e_t")
                    s_sum = small.tile([P, 1], f32, tag="s_sum")
                    nc.scalar.activation(
                        e_t[:], logits[:], Fn.Exp, accum_out=s_sum[:]
                    )
                    logs = small.tile([P, 1], f32, tag="logs")
                    nc.scalar.activation(logs[:], s_sum[:], Fn.Ln)
                    neglogs = small.tile([P, 1], f32, tag="neglogs")
                    nc.gpsimd.tensor_scalar_mul(neglogs[:], logs[:], -1.0)
                    nc.scalar.activation(
                        enc_sb[:, g, :], logits[:], Fn.Exp, bias=neglogs[:], scale=1.0
                    )

                    # encT (4 PE transposes into one PSUM bank) + escape
                    encT_ps = psum.tile([P, C], f32, tag="encT_ps")
                    for k in range(4):
                        nc.tensor.matmul(
                            encT_ps[:, k * P : (k + 1) * P],
                            enc_sb[:, g, k * P : (k + 1) * P],
                            id_s[:],
                            is_transpose=True, start=(k == 0), stop=(k == 3),
                        )
                    encT_sb = work.tile([P, C], f32, tag="encT_sb")
                    nc.scalar.copy(encT_sb[:], encT_ps[:])

                    # q = enc @ cb  (4 accumulating matmuls over code chunks)
                    q_ps = psum.tile([P, D], f32, tag="q_ps")
                    for k in range(4):
                        nc.tensor.matmul(
                            q_ps[:],
                            encT_sb[:, k * P : (k + 1) * P],
                            cb_s[:, k, :],
                            start=(k == 0), stop=(k == 3),
                        )
                    nc.vector.tensor_copy(q_sb[:, g, :], q_ps[:])

                nc.sync.dma_start(
                    q_d[tok, :].rearrange("(g p) d -> p g d", p=P), q_sb[:]
                )
                nc.sync.dma_start(
                    enc_d[tok, :].rearrange("(g p) c -> p g c", p=P), enc_sb[:]
                )
                nc.sync.dma_start(
                    idx_d[tok].rearrange("(g p) -> p g", p=P), idx_sb[:]
                )

    nc.compile()
    return nc


def make_in_maps(x, mask, codebook, noise):
    """Shard full inputs into per-core input maps."""
    x = np.ascontiguousarray(x, dtype=np.float32)
    mask = np.ascontiguousarray(mask, dtype=np.float32)
    noise = np.ascontiguousarray(noise, dtype=np.float32)
    cb = np.ascontiguousarray(codebook, dtype=np.float32)
    cbT = np.ascontiguousarray(cb.T)
    ident = np.eye(P, dtype=np.float32)
    n = x.shape[0]
    r = n // N_CORES
    in_maps = []
    for c in range(N_CORES):
        sl = slice(c * r, (c + 1) * r)
        in_maps.append(
            {
                "x": x[sl],
                "mask": mask[sl],
                "noise": noise[sl],
                "cbT": cbT,
                "cb": cb,
                "ident": ident,
            }
        )
    return in_maps


def kernel(x, mask, codebook, noise):
    from concourse.bass_utils import run_bass_kernel_spmd

    n = x.shape[0]
    r = n // N_CORES
    if r not in _CACHE:
        _CACHE[r] = build_nc(r)
    nc = _CACHE[r]

    in_maps = make_in_maps(x, mask, codebook, noise)
    res = run_bass_kernel_spmd(nc, in_maps, list(range(N_CORES)))

    quantized = np.concatenate([res.results[c]["quantized"] for c in range(N_CORES)], axis=0)
    encodings = np.concatenate([res.results[c]["encodings"] for c in range(N_CORES)], axis=0)
    indices = np.concatenate([res.results[c]["indices"] for c in range(N_CORES)], axis=0)
    return quantized, encodings, indices.astype(np.int32)
